# revision 55
# baseline (speedup 1.0000x reference)
"""Multi-head attention on 8 TRN2 NeuronCores (data/head-parallel).

Problem: B=4 H=16 S=2048 D=64 fp32 attention, out = softmax(Q K^T / sqrt(D)) V.
B*H = 64 (batch, head) pairs are sharded 8-per-core; each core runs the same
NEFF over its own 8 heads, no collectives.

Host-side prep (part of sharding): Q and K are transposed to [D, S] and cast
to bf16, V gets a ones column appended (giving softmax denominators for free
out of the P@V matmul) and is cast to bf16. The device then runs, per head:

  - S^T[k, q] = K^T.T @ Q^T on PE (contraction over d=64, bf16, fp32 PSUM).
  - E^T = exp(S^T / sqrt(D)) on ACT (PSUM -> SBUF bf16); the 1/sqrt(D) scale
    rides the activation's free affine input scale.
  - out'^T[d', q] = sum_k V'[k, d'].T @ E^T[k, q] accumulated in PSUM, where
    V' row 64 (ones) accumulates the softmax denominators.
  - PE transposes out'^T back to [q, 65] tiles; DVE takes reciprocals of the
    denominator column and scales; DMA out f32.
"""

import math
from contextlib import ExitStack

import ml_dtypes
import numpy as np

import concourse.bass as bass
import concourse.bacc as bacc
import concourse.tile as tile
import concourse.mybir as mybir
from concourse.bass_utils import run_bass_kernel_spmd
from concourse.masks import make_identity

B, H, S, D = 4, 16, 2048, 64
N_CORES = 8
HPC = B * H // N_CORES     # heads per core
ST = S // 128              # 16 s-tiles of 128
QCHUNK = 1024              # q processed in chunks (PSUM budget)
NQ = S // QCHUNK
MMN = 512                  # moving free dim per matmul (one PSUM bank)
DT = mybir.dt

_BUILT = {}


class _Bacc(bacc.Bacc):
    """Bacc with the move-matmul-waits-to-ldweights pass disabled: keeping
    waits on the matmul (not its LDWEIGHTS) lets the PE queue pull weight
    loads ahead of in-flight matmuls, hiding the ~70ns LDW cost."""

    def move_matmul_waits_to_ldweights(self):
        pass


def _head(nc, pools, id65, scale, qt_d, kt_d, vp_d, o_d, h):
    (stage, epool, spool, outp, ps_st, ps_ot, ps_tt) = pools

    # ---- loads (bf16, pre-transposed + zero-padded to 128 on host) -------
    # Stationary operands must be 128x128 for the compiler to enable FWL
    # (fast weight load); smaller loads serialize ~110ns per matmul pair.
    qt = stage.tile([128, S], DT.bfloat16, tag="qt")
    kt = stage.tile([128, S], DT.bfloat16, tag="kt")
    vp = stage.tile([128, ST, 128], DT.bfloat16, tag="vp")
    for j in range(4):
        quarter = slice(j * (S // 4), (j + 1) * (S // 4))
        # Head 0's first quarters issue from Scalar's HWDGE (idle until the
        # first exp) in parallel with GpSimd, shortening the cold prologue.
        eng = nc.scalar if (h == 0 and j <= 1) else nc.gpsimd
        eng.dma_start(out=qt[:, quarter], in_=qt_d[h][:, quarter])
        eng.dma_start(out=kt[:, quarter], in_=kt_d[h][:, quarter])
    vp_v = vp_d[h].rearrange("(t p) e -> p t e", p=128)
    for j in range(2):
        sl = slice(8 * j, 8 * j + 8)
        nc.gpsimd.dma_start(out=vp[:, sl, :], in_=vp_v[:, sl, :])

    # ---- attention per q-chunk -------------------------------------------
    for c in range(NQ):
        q0 = c * QCHUNK
        ets = []
        for t in range(ST):
            st = ps_st.tile([128, QCHUNK], DT.float32, tag="st")
            for n in range(QCHUNK // MMN):
                nc.tensor.matmul(
                    st[:, n * MMN : (n + 1) * MMN],
                    lhsT=kt[:, t * 128 : (t + 1) * 128],
                    rhs=qt[:, q0 + n * MMN : q0 + (n + 1) * MMN],
                    start=True,
                    stop=True,
                )
            et = epool.tile([128, QCHUNK], DT.bfloat16, tag=f"et{t}")
            nc.scalar.activation(
                out=et, in_=st, func=mybir.ActivationFunctionType.Exp, scale=scale
            )
            ets.append(et)

        ot = ps_ot.tile([128, QCHUNK], DT.float32, tag="ot")
        for t in range(ST):
            for n in range(QCHUNK // MMN):
                nc.tensor.matmul(
                    ot[:, n * MMN : (n + 1) * MMN],
                    lhsT=vp[:, t, :],
                    rhs=ets[t][:, n * MMN : (n + 1) * MMN],
                    start=(t == 0),
                    stop=(t == ST - 1),
                )

        # ---- normalize: transpose back, scale by 1/denominator ----------
        ots = spool.tile([D + 1, QCHUNK], DT.float32, tag="ots")
        for half in range(2):
            hs = slice(half * (QCHUNK // 2), (half + 1) * (QCHUNK // 2))
            nc.vector.tensor_copy(out=ots[:, hs], in_=ot[0 : D + 1, hs])
        outst = outp.tile([128, QCHUNK // 128, D], DT.float32, tag="outst")
        o_v = o_d[h, q0 : q0 + QCHUNK, :].rearrange("(r p) d -> p r d", p=128)
        nquad = QCHUNK // (4 * 128)
        for g in range(nquad):
            tt = ps_tt.tile([128, 4 * (D + 1)], DT.float32, tag="tt")
            for j in range(4):
                r = 4 * g + j
                nc.tensor.transpose(
                    tt[:, j * (D + 1) : (j + 1) * (D + 1)],
                    ots[:, r * 128 : (r + 1) * 128],
                    id65,
                )
            ttv = tt.rearrange("p (j x) -> p j x", j=4)
            rec = spool.tile([128, 4], DT.float32, tag="rec")
            nc.vector.reciprocal(out=rec, in_=ttv[:, :, D])
            for j in range(4):
                nc.vector.tensor_scalar(
                    outst[:, 4 * g + j, :],
                    ttv[:, j, 0:D],
                    rec[:, j : j + 1],
                    None,
                    mybir.AluOpType.mult,
                )
            # store this quad as soon as it is normalized; Sync's HWDGE is
            # otherwise idle, so store issue never queues behind loads
            sl = slice(4 * g, 4 * g + 4)
            nc.sync.dma_start(out=o_v[:, sl, :], in_=outst[:, sl, :])


def build_graph(scale: float, heads: int = HPC):
    nc = _Bacc("TRN2", target_bir_lowering=False, debug=False,
               num_devices=N_CORES)
    qt_d = nc.dram_tensor("QT", [heads, 128, S], DT.bfloat16,
                          kind="ExternalInput").ap()
    kt_d = nc.dram_tensor("KT", [heads, 128, S], DT.bfloat16,
                          kind="ExternalInput").ap()
    vp_d = nc.dram_tensor("VP", [heads, S, 128], DT.bfloat16,
                          kind="ExternalInput").ap()
    id_d = nc.dram_tensor("ID", [D + 1, D + 1], DT.float32,
                          kind="ExternalInput").ap()
    o_d = nc.dram_tensor("out", [heads, S, D], DT.float32,
                         kind="ExternalOutput").ap()

    with tile.TileContext(nc) as tc, ExitStack() as ctx:
        const = ctx.enter_context(tc.tile_pool(name="const", bufs=1))
        stage = ctx.enter_context(tc.tile_pool(name="stage", bufs=3))
        epool = ctx.enter_context(tc.tile_pool(name="epool", bufs=3))
        spool = ctx.enter_context(tc.tile_pool(name="spool", bufs=2))
        outp = ctx.enter_context(tc.tile_pool(name="outp", bufs=2))
        ps_st = ctx.enter_context(tc.tile_pool(name="ps_st", bufs=2, space="PSUM"))
        ps_ot = ctx.enter_context(tc.tile_pool(name="ps_ot", bufs=1, space="PSUM"))
        ps_tt = ctx.enter_context(tc.tile_pool(name="ps_tt", bufs=2, space="PSUM"))

        id65 = const.tile([D + 1, D + 1], DT.float32)
        nc.sync.dma_start(out=id65, in_=id_d)

        pools = (stage, epool, spool, outp, ps_st, ps_ot, ps_tt)
        for h in range(heads):
            _head(nc, pools, id65, scale, qt_d, kt_d, vp_d, o_d, h)

    nc.compile()
    return nc


def _get_nc(scale: float):
    key = round(float(scale), 9)
    if key not in _BUILT:
        _BUILT[key] = build_graph(float(scale))
    return _BUILT[key]


def shard_inputs(Q, K, V):
    """Host-side prep: shard heads across cores, pre-transpose Q/K to [D,S]
    bf16, append a ones column to V (bf16)."""
    bf16 = ml_dtypes.bfloat16
    qs = np.asarray(Q, dtype=np.float32).reshape(B * H, S, D)
    ks = np.asarray(K, dtype=np.float32).reshape(B * H, S, D)
    vs = np.asarray(V, dtype=np.float32).reshape(B * H, S, D)
    qt = np.zeros((B * H, 128, S), dtype=bf16)
    kt = np.zeros((B * H, 128, S), dtype=bf16)
    qt[:, :D, :] = qs.transpose(0, 2, 1).astype(bf16)
    kt[:, :D, :] = ks.transpose(0, 2, 1).astype(bf16)
    vp = np.zeros((B * H, S, 128), dtype=bf16)
    vp[:, :, :D] = vs.astype(bf16)
    vp[:, :, D] = np.float32(1.0)
    eye = np.eye(D + 1, dtype=np.float32)
    in_maps = []
    for c in range(N_CORES):
        sl = slice(c * HPC, (c + 1) * HPC)
        in_maps.append({
            "QT": np.ascontiguousarray(qt[sl]),
            "KT": np.ascontiguousarray(kt[sl]),
            "VP": np.ascontiguousarray(vp[sl]),
            "ID": eye,
        })
    return in_maps


def kernel(Q, K, V, d_k, **run_kwargs):
    scale = 1.0 / math.sqrt(float(d_k))
    nc = _get_nc(scale)
    in_maps = shard_inputs(Q, K, V)
    res = run_bass_kernel_spmd(nc, in_maps, core_ids=list(range(N_CORES)),
                               **run_kwargs)
    out = np.concatenate([r["out"] for r in res.results], axis=0)
    out = out.reshape(B, H, S, D).astype(np.float32)
    kernel.last_results = res
    return out
